# revision 5
# baseline (speedup 1.0000x reference)
"""Trainium2 Bass kernel for nn_CAModel (neural cellular automaton step).

Per-core (8-way batch-parallel, 2 images/core) bf16 pipeline, v2:
  - packed layout: partition p = u*16 + c  (u = row-block of 24 rows, c = channel)
  - depthwise sobel convs built separably (img0 on DVE for fast ramp, img1 on Pool)
  - MLP per (img, 384-px tile): L1 = 2 parity sets x 3 k-rounds of 4 concurrent
    quadrant matmuls (K=32, tile_position rows), L2 dense K=128, L3 col-tiled
    2 rounds of 4; PSUM plan 4+2+2 banks so the PE stream stays dense
  - x_new = (z3+b3)*umask + x fused into the per-tile evacuation (no epilogue tail)
  - relu evacuation split ACT/DVE by pattern (relu6 == relu here: preacts < 6)
  - life masks via stripe-packed 3x3 maxpool; bf16 output, one store DMA per image
"""

import numpy as np
import ml_dtypes
import concourse.bass as bass
import concourse.tile as tile
from concourse import bacc, mybir

AF = mybir.ActivationFunctionType
OP = mybir.AluOpType
f16 = mybir.dt.bfloat16
f32 = mybir.dt.float32

BL, C, H, W = 2, 16, 192, 192   # per-core images
U, RPU = 8, 24                  # row-block units per image, rows per unit
FPI = RPU * W                   # 4608 free elems per (img,unit)
FHI = (RPU + 2) * W             # 4992 per img in halo'd layout
NT, TS = 12, 384                # tiles per (img,unit), pixels per tile
HHF = 12 * W                    # 2304 free elems per dw half-tile
HID = 128

# relu-evac engine split: index by u for h1/h2 ('a' = ACT, 'v' = DVE)
EV1 = "aavaaava"
EV2 = "aavaavaa"
WARMN = 70  # PE warmup matmuls to cover the initial DMA + dw-build runway


def build_nc():
    nc = bacc.Bacc("TRN2", target_bir_lowering=False, debug=False)

    x_d = nc.dram_tensor("x", [BL, C, H, W], f16, kind="ExternalInput")
    fn_d = nc.dram_tensor("fn", [BL, H, W], f16, kind="ExternalInput")  # host-side umask {0,1}
    wstack_d = nc.dram_tensor("wstack", [128, 768], f16, kind="ExternalInput")
    w2t_d = nc.dram_tensor("w2t", [128, 128], f16, kind="ExternalInput")
    w3t_d = nc.dram_tensor("w3t", [128, 64], f16, kind="ExternalInput")
    b1_d = nc.dram_tensor("b1", [128, 1], f32, kind="ExternalInput")
    b2_d = nc.dram_tensor("b2", [128, 1], f32, kind="ExternalInput")
    b3_d = nc.dram_tensor("b3", [128, 1], f32, kind="ExternalInput")
    out_d = nc.dram_tensor("out", [BL, C, H, W], f16, kind="ExternalOutput")

    with tile.TileContext(nc) as tc:
        with (
            tc.tile_pool(name="const", bufs=1) as const,
            tc.tile_pool(name="xf", bufs=1) as xfp,
            tc.tile_pool(name="dw", bufs=1) as dwp,
            tc.tile_pool(name="chk", bufs=2) as chk,
            tc.tile_pool(name="msk", bufs=1) as mskp,
            tc.tile_pool(name="strp", bufs=1) as strp,
            tc.tile_pool(name="h1p", bufs=9) as h1p,
            tc.tile_pool(name="h2p", bufs=10) as h2p,
            tc.tile_pool(name="dram", bufs=1, space="DRAM") as dramp,
            tc.tile_pool(name="pz1", bufs=4, space="PSUM") as pz1,
            tc.tile_pool(name="pz2", bufs=2, space="PSUM") as pz2,
            tc.tile_pool(name="pz3", bufs=2, space="PSUM") as pz3,
        ):
            # ---- constants ----
            wstack = const.tile([128, 768], f16)
            nc.sync.dma_start(wstack[:], wstack_d.ap())
            w2t = const.tile([128, 128], f16)
            nc.sync.dma_start(w2t[:], w2t_d.ap())
            w3t = const.tile([128, 64], f16)
            nc.sync.dma_start(w3t[:], w3t_d.ap())
            b1c = const.tile([128, 1], f32)
            nc.sync.dma_start(b1c[:], b1_d.ap())
            b2c = const.tile([128, 1], f32)
            nc.sync.dma_start(b2c[:], b2_d.ap())
            b3c = const.tile([128, 1], f32)
            nc.sync.dma_start(b3c[:], b3_d.ap())

            # ---- load x bf16 (halo'd rows: buffer row r -> image row u*24 + r - 1) ----
            xf = xfp.tile([128, BL, RPU + 2, W], f16)
            nc.vector.memset(xf[0:32, :, 0:1, :], 0.0)
            nc.vector.memset(xf[96:128, :, 25:26, :], 0.0)
            for img in range(BL):
                for u in range(U):
                    lo = max(0, u * RPU - 1)
                    hi = min(H, u * RPU + RPU + 1)
                    rb0 = 1 - (u * RPU - lo)  # 0 normally; 1 for u==0
                    nc.sync.dma_start(
                        xf[u * 16:(u + 1) * 16, img, rb0:rb0 + (hi - lo), :],
                        x_d.ap()[img, :, lo:hi, :],
                    )

            # ---- update mask (host-computed {0,1}), broadcast over channels ----
            umasks = [mskp.tile([128, RPU, W], f16, tag=f"um{i}", name=f"um{i}") for i in range(BL)]
            for img in range(BL):
                for u in range(U):
                    src = fn_d.ap()[img, u * RPU:(u + 1) * RPU, :]
                    src = src.rearrange("a b -> (a b)").partition_broadcast(16)
                    nc.sync.dma_start(umasks[img][u * 16:(u + 1) * 16], src)

            # ---- PE warmup runway over the DMA/dw-build ramp ----
            zw = pz3.tile([128, TS], f32, tag="z3", name="zw")
            for _ in range(WARMN):
                nc.tensor.matmul(zw[:, :], w2t[:, :], wstack[:, 0:TS], start=True, stop=True)

            # ---- depthwise sobel builds (separable, pairwise sums) ----
            # per img: 2 half tiles (rows 0:12, 12:24) per tensor so L1 can start
            # after the first half. img0 on DVE (fast ramp), img1 on Pool (overlaps).
            dwxs_t = [[dwp.tile([128, 12, W], f16, tag=f"dwx{i}{h}", name=f"dwx{i}{h}")
                       for h in range(2)] for i in range(BL)]
            dwys_t = [[dwp.tile([128, 12, W], f16, tag=f"dwy{i}{h}", name=f"dwy{i}{h}")
                       for h in range(2)] for i in range(BL)]
            RC = 6  # chunk rows

            def emit_build(img, eng):
                for r0 in range(0, RPU, RC):
                    # x buffer rows r0 .. r0+RC+2 cover image rows r0-1 .. r0+RC+1
                    ps = chk.tile([128, RC + 1, W], f16, tag="ps")
                    eng.tensor_add(
                        ps[:], xf[:, img, r0:r0 + RC + 1, :], xf[:, img, r0 + 1:r0 + RC + 2, :]
                    )
                    v1 = chk.tile([128, RC, W], f16, tag="v1")
                    eng.tensor_add(v1[:], ps[:, 0:RC, :], ps[:, 1:RC + 1, :])
                    v2 = chk.tile([128, RC, W], f16, tag="v2")
                    eng.tensor_sub(
                        v2[:], xf[:, img, r0 + 2:r0 + RC + 2, :], xf[:, img, r0:r0 + RC, :]
                    )
                    qs = chk.tile([128, RC, W], f16, tag="qs")
                    eng.tensor_add(qs[:, :, 0:191], v2[:, :, 0:191], v2[:, :, 1:192])
                    rr = r0 % 12
                    dxs = dwxs_t[img][r0 // 12][:, rr:rr + RC, :]
                    dys = dwys_t[img][r0 // 12][:, rr:rr + RC, :]
                    # dwx = v1[c+1] - v1[c-1]; borders zero-padded
                    eng.tensor_sub(dxs[:, :, 1:191], v1[:, :, 2:192], v1[:, :, 0:190])
                    eng.tensor_copy(dxs[:, :, 0:1], v1[:, :, 1:2])
                    eng.tensor_scalar_mul(dxs[:, :, 191:192], v1[:, :, 190:191], -1.0)
                    # dwy = qs[c-1] + qs[c]; borders: qs[0]+v2[0], qs[190]+v2[191]
                    eng.tensor_add(dys[:, :, 1:191], qs[:, :, 0:190], qs[:, :, 1:191])
                    eng.tensor_add(dys[:, :, 0:1], qs[:, :, 0:1], v2[:, :, 0:1])
                    eng.tensor_add(dys[:, :, 191:192], qs[:, :, 190:191], v2[:, :, 191:192])

            emit_build(0, nc.vector)
            emit_build(1, nc.gpsimd)

            # ---- pre-life maxpool (stripe layout: partition = img*64 + s, 3 rows each) ----
            xf_r = xf[:].rearrange("(u c) i r w -> u c i r w", c=16)

            al = strp.tile([128, 3, W], f16, tag="al_s")
            for img in range(BL):
                s = xf_r[:, 3, img, 1:25, :]
                s = s.rearrange("u (s r) w -> u s (r w)", r=3)
                nc.sync.dma_start(al[img * 64:(img + 1) * 64, :, :], s)
            pm = strp.tile([128, 3, 191], f16, tag="pm_s")
            nc.vector.tensor_max(pm[:], al[:, :, 0:191], al[:, :, 1:192])
            m1 = strp.tile([128, 3, W], f16, tag="m1_s")
            nc.vector.tensor_max(m1[:, :, 1:191], pm[:, :, 0:190], pm[:, :, 1:191])
            nc.vector.tensor_copy(m1[:, :, 0:1], pm[:, :, 0:1])
            nc.vector.tensor_copy(m1[:, :, 191:192], pm[:, :, 190:191])
            hh = strp.tile([128, 2, W], f16, tag="hh_s")
            nc.sync.dma_start(hh[1:128, 0, :], m1[0:127, 2, :])
            nc.sync.dma_start(hh[0:127, 1, :], m1[1:128, 0, :])
            m1_r = m1[:].rearrange("(i s) r w -> i s r w", s=64)
            hh_r = hh[:].rearrange("(i s) r w -> i s r w", s=64)
            nc.sync.dma_start(hh_r[:, 0, 0, :], m1_r[:, 0, 0, :])
            nc.sync.dma_start(hh_r[:, 63, 1, :], m1_r[:, 63, 2, :])
            pv = strp.tile([128, 2, W], f16, tag="pv_s")
            nc.vector.tensor_max(pv[:], m1[:, 0:2, :], m1[:, 1:3, :])
            m2pre = strp.tile([128, 3, W], f16, tag="m2_pre")
            nc.vector.tensor_max(m2pre[:, 0, :], pv[:, 0, :], hh[:, 0, :])
            nc.vector.tensor_max(m2pre[:, 1, :], pv[:, 0, :], pv[:, 1, :])
            nc.vector.tensor_max(m2pre[:, 2, :], pv[:, 1, :], hh[:, 1, :])

            # ---- MLP over tiles ----
            xf_flat = xf[:].rearrange("p i r w -> p (i r w)")
            dwx_flats = [[t[:].rearrange("p r w -> p (r w)") for t in hs] for hs in dwxs_t]
            dwy_flats = [[t[:].rearrange("p r w -> p (r w)") for t in hs] for hs in dwys_t]
            xnews = [mskp.tile([128, RPU, W], f16, tag=f"xn{i}", name=f"xn{i}") for i in range(BL)]
            xn_flats = [t[:].rearrange("p r w -> p (r w)") for t in xnews]
            um_flats = [t[:].rearrange("p r w -> p (r w)") for t in umasks]

            def mlp_tile(img, t):
                off = t * TS
                h = off // HHF
                hoff = off - h * HHF
                h1s = {}
                # L1: parity sets; critical runs keep same-weight quadrant MMs
                # back-to-back (LDW pull-ahead + array concurrency + HAM warmth)
                for par in range(2):
                    z1s = {u: pz1.tile([128, TS], f32, tag="z1", name=f"z1_{img}_{t}_{u}")
                           for u in (2 * q + par for q in range(4))}
                    with tc.tile_critical():
                        for k in range(3):
                            for q in range(4):
                                u = 2 * q + par
                                base = 32 * q
                                blk = 2 * k + par
                                wv = wstack[base:base + 32, blk * 128:(blk + 1) * 128]
                                if k == 0:
                                    flat = xf_flat
                                    o = img * FHI + W + off
                                else:
                                    flat = (dwx_flats if k == 1 else dwy_flats)[img][h]
                                    o = hoff
                                nc.tensor.matmul(
                                    z1s[u][:, :], wv, flat[base:base + 32, o:o + TS],
                                    start=(k == 0), stop=(k == 2), tile_position=(base, 0),
                                )
                    for q in range(4):
                        u = 2 * q + par
                        h1g = h1p.tile([128, TS], f16, tag="h1", name=f"h1_{img}_{t}_{u}")
                        if EV1[u] == "a":
                            nc.scalar.activation(h1g[:], z1s[u][:], AF.Relu, bias=b1c[:])
                        else:
                            nc.vector.tensor_scalar(h1g[:], z1s[u][:], b1c[:], 0.0, OP.add, OP.max)
                        h1s[u] = h1g
                # L2: dense K=128 per unit; sub-runs of 2 (z2 double-buffered)
                h2s = {}
                for u0 in range(0, U, 2):
                    z2a = pz2.tile([128, TS], f32, tag="z2", name=f"z2_{img}_{t}_{u0}")
                    z2b = pz2.tile([128, TS], f32, tag="z2", name=f"z2_{img}_{t}_{u0 + 1}")
                    with tc.tile_critical():
                        nc.tensor.matmul(z2a[:, :], w2t[:, :], h1s[u0][:], start=True, stop=True)
                        nc.tensor.matmul(z2b[:, :], w2t[:, :], h1s[u0 + 1][:], start=True, stop=True)
                    for u, z2g in ((u0, z2a), (u0 + 1, z2b)):
                        h2g = h2p.tile([128, TS], f16, tag="h2", name=f"h2_{img}_{t}_{u}")
                        if EV2[u] == "a":
                            nc.scalar.activation(h2g[:], z2g[:], AF.Relu, bias=b2c[:])
                        else:
                            nc.vector.tensor_scalar(h2g[:], z2g[:], b2c[:], 0.0, OP.add, OP.max)
                        h2s[u] = h2g
                # L3: one critical run; 2 rounds of 4 col-group matmuls into one bank
                z3 = pz3.tile([128, TS], f32, tag="z3", name=f"z3_{img}_{t}")
                with tc.tile_critical():
                    for par in range(2):
                        for j in range(4):
                            nc.tensor.matmul(
                                z3[32 * j:32 * j + 32, :], w3t[:, 32 * par:32 * par + 32],
                                h2s[2 * j + par][:], start=(par == 0), stop=(par == 1),
                                tile_position=(0, 32 * j),
                            )
                # fused epilogue: x_new = (z3 + b3) * umask + x
                xn_sl = xn_flats[img][:, off:off + TS]
                nc.vector.scalar_tensor_tensor(
                    xn_sl, z3[:, :], b3c[:], um_flats[img][:, off:off + TS], OP.add, OP.mult
                )
                nc.gpsimd.tensor_add(xn_sl, xn_sl, xf_flat[:, img * FHI + W + off:img * FHI + W + off + TS])

            # ---- per-image post: post-life maxpool + life mask + store ----
            lifec_d = dramp.tile([128, 3 * W], f16)
            al_post = strp.tile([128, 3, W], f16, tag="al_p")
            pm_post = strp.tile([128, 3, 191], f16, tag="pm_p")
            m1_post = strp.tile([128, 3, W], f16, tag="m1_p")
            hh_post = strp.tile([128, 2, W], f16, tag="hh_p")
            pv_post = strp.tile([128, 2, W], f16, tag="pv_p")
            m2_post = strp.tile([128, 3, W], f16, tag="m2_p")
            lifec = strp.tile([128, 3 * W], f16, tag="lifec")

            def emit_post(img):
                xni = xnews[img]
                xn_r = xni[:].rearrange("(u c) r w -> u c r w", c=16)
                src = xn_r[:, 3, :, :].rearrange("u (s r) w -> u s (r w)", r=3)
                sl = slice(img * 64, (img + 1) * 64)
                nc.sync.dma_start(al_post[sl, :, :], src)
                nc.vector.tensor_max(pm_post[sl], al_post[sl, :, 0:191], al_post[sl, :, 1:192])
                nc.vector.tensor_max(m1_post[sl, :, 1:191], pm_post[sl, :, 0:190], pm_post[sl, :, 1:191])
                nc.vector.tensor_copy(m1_post[sl, :, 0:1], pm_post[sl, :, 0:1])
                nc.vector.tensor_copy(m1_post[sl, :, 191:192], pm_post[sl, :, 190:191])
                nc.sync.dma_start(hh_post[img * 64 + 1:(img + 1) * 64, 0, :], m1_post[img * 64:(img + 1) * 64 - 1, 2, :])
                nc.sync.dma_start(hh_post[img * 64:(img + 1) * 64 - 1, 1, :], m1_post[img * 64 + 1:(img + 1) * 64, 0, :])
                nc.sync.dma_start(hh_post[img * 64:img * 64 + 1, 0, :], m1_post[img * 64:img * 64 + 1, 0, :])
                nc.sync.dma_start(hh_post[(img + 1) * 64 - 1:(img + 1) * 64, 1, :], m1_post[(img + 1) * 64 - 1:(img + 1) * 64, 2, :])
                nc.vector.tensor_max(pv_post[sl], m1_post[sl, 0:2, :], m1_post[sl, 1:3, :])
                nc.vector.tensor_max(m2_post[sl, 0, :], pv_post[sl, 0, :], hh_post[sl, 0, :])
                nc.vector.tensor_max(m2_post[sl, 1, :], pv_post[sl, 0, :], pv_post[sl, 1, :])
                nc.vector.tensor_max(m2_post[sl, 2, :], pv_post[sl, 1, :], hh_post[sl, 1, :])

                nc.vector.tensor_tensor(
                    lifec[sl], m2pre[:].rearrange("p r w -> p (r w)")[sl],
                    m2_post[:].rearrange("p r w -> p (r w)")[sl], OP.min,
                )
                nc.vector.tensor_scalar(lifec[sl], lifec[sl], 0.1, None, OP.is_gt)

                # broadcast life over channels (bounce via DRAM)
                nc.sync.dma_start(lifec_d[sl], lifec[sl])
                life = mskp.tile([128, RPU, W], f16, tag=f"life{img}", name=f"life{img}")
                for u in range(U):
                    bsrc = lifec_d[img * 64 + 8 * u: img * 64 + 8 * u + 8, :]
                    bsrc = bsrc.rearrange("s w -> (s w)").partition_broadcast(16)
                    nc.sync.dma_start(life[u * 16:(u + 1) * 16], bsrc)

                # final mask multiply (bf16 out) + one big store
                out16 = mskp.tile([128, RPU, W], f16, tag=f"o16{img}", name=f"o16{img}")
                nc.vector.tensor_mul(out16[:], xni[:], life[:])
                for u in range(U):
                    nc.sync.dma_start(
                        out_d.ap()[img, :, u * RPU:(u + 1) * RPU, :],
                        out16[u * 16:(u + 1) * 16],
                    )

            # ---- driver ----
            for img in range(BL):
                for t in range(NT):
                    mlp_tile(img, t)
                emit_post(img)

    nc.compile()
    return nc


def host_prep(inputs):
    """Full inputs -> list of 8 per-core input dicts."""
    x = np.ascontiguousarray(inputs["x"], dtype=np.float32)
    fn = np.ascontiguousarray(inputs["fire_noise"], dtype=np.float32)
    w1 = np.asarray(inputs["w1"], np.float32)
    b1 = np.asarray(inputs["b1"], np.float32)
    w2 = np.asarray(inputs["w2"], np.float32)
    b2 = np.asarray(inputs["b2"], np.float32)
    w3 = np.asarray(inputs["w3"], np.float32)
    b3 = np.asarray(inputs["b3"], np.float32)

    w1a, w1b, w1c = w1[:, 0:16], w1[:, 16:32] / 8.0, w1[:, 32:48] / 8.0
    wstack = np.zeros((128, 768), ml_dtypes.bfloat16)
    for g in range(4):
        for k, comp in enumerate((w1a, w1b, w1c)):
            for par in range(2):
                blk = 2 * k + par
                r0 = 32 * g + 16 * par
                wstack[r0:r0 + 16, blk * 128:(blk + 1) * 128] = comp.T.astype(ml_dtypes.bfloat16)
    w2t = w2.T.astype(ml_dtypes.bfloat16)
    w3t = np.zeros((128, 64), ml_dtypes.bfloat16)
    w3t[:, 0:16] = w3.T.astype(ml_dtypes.bfloat16)
    w3t[:, 48:64] = w3.T.astype(ml_dtypes.bfloat16)
    b3col = np.tile(b3, U).reshape(128, 1).astype(np.float32)

    shared = {
        "wstack": wstack, "w2t": w2t, "w3t": w3t,
        "b1": b1.reshape(128, 1).astype(np.float32),
        "b2": b2.reshape(128, 1).astype(np.float32),
        "b3": b3col,
    }
    xh = x.astype(ml_dtypes.bfloat16)
    um = (fn[:, 0] <= 0.5).astype(ml_dtypes.bfloat16)
    in_maps = []
    for core in range(8):
        m = dict(shared)
        m["x"] = xh[2 * core:2 * core + 2]
        m["fn"] = um[2 * core:2 * core + 2]
        in_maps.append(m)
    return in_maps


_NC_CACHE = None


def kernel(**inputs):
    global _NC_CACHE
    from concourse.bass_utils import run_bass_kernel_spmd
    if _NC_CACHE is None:
        _NC_CACHE = build_nc()
    in_maps = host_prep(inputs)
    res = run_bass_kernel_spmd(_NC_CACHE, in_maps, core_ids=list(range(8)))
    return np.concatenate(
        [np.asarray(res.results[i]["out"]).astype(np.float32) for i in range(8)], axis=0
    )


# revision 6
# speedup vs baseline: 2.0295x; 2.0295x over previous
"""Trainium2 Bass kernel for nn_CAModel (neural cellular automaton step).

Per-core (8-way batch-parallel, 2 images/core) bf16 pipeline:
  - packed layout: partition p = u*16 + c  (u = row-block of 24 rows, c = channel)
  - depthwise sobel convs built separably (img0 on DVE for fast ramp, img1 on Pool)
  - MLP per (img, 512-px tile): L1 = 2 parity sets x 3 k-rounds of 4 quadrant
    matmuls (K=32, tile_position rows), L2 dense K=128, L3 col-tiled 2 rounds
    of 4; PSUM plan 4+2+2 single-bank tiles keeps the PE stream dense
  - x_new = (z3+b3)*umask + x fused into the per-tile evacuation (no epilogue tail)
  - relu evacuation split ACT/DVE by pattern (relu6 == relu here: preacts < 6)
  - life masks via stripe-packed 3x3 maxpool; bf16 output stores
"""

import numpy as np
import ml_dtypes
import concourse.bass as bass
import concourse.tile as tile
from concourse import bacc, mybir

AF = mybir.ActivationFunctionType
OP = mybir.AluOpType
f16 = mybir.dt.bfloat16
f32 = mybir.dt.float32

BL, C, H, W = 2, 16, 192, 192   # per-core images
U, RPU = 8, 24                  # row-block units per image, rows per unit
FPI = RPU * W                   # 4608 free elems per (img,unit)
FHI = (RPU + 2) * W             # 4992 per img in halo'd layout
NT, TS = 9, 512                 # tiles per (img,unit), pixels per tile
HID = 128

# relu-evac engine split: index by u for h1/h2 ('a' = ACT, 'v' = DVE)
EV1 = "aavaavav"
EV2 = "aavaavaa"
WARMN = 60  # PE warmup matmuls to cover the initial DMA + dw-build runway


def build_nc():
    nc = bacc.Bacc("TRN2", target_bir_lowering=False, debug=False)

    x_d = nc.dram_tensor("x", [BL, C, H, W], f16, kind="ExternalInput")
    fn_d = nc.dram_tensor("fn", [BL, H, W], f16, kind="ExternalInput")  # host-side umask {0,1}
    wstack_d = nc.dram_tensor("wstack", [128, 768], f16, kind="ExternalInput")
    w2t_d = nc.dram_tensor("w2t", [128, 128], f16, kind="ExternalInput")
    w3t_d = nc.dram_tensor("w3t", [128, 64], f16, kind="ExternalInput")
    b1_d = nc.dram_tensor("b1", [128, 1], f32, kind="ExternalInput")
    b2_d = nc.dram_tensor("b2", [128, 1], f32, kind="ExternalInput")
    b3_d = nc.dram_tensor("b3", [128, 1], f32, kind="ExternalInput")
    out_d = nc.dram_tensor("out", [BL, C, H, W], f16, kind="ExternalOutput")

    with tile.TileContext(nc) as tc:
        with (
            tc.tile_pool(name="const", bufs=1) as const,
            tc.tile_pool(name="xf", bufs=1) as xfp,
            tc.tile_pool(name="dw", bufs=1) as dwp,
            tc.tile_pool(name="chk", bufs=2) as chk,
            tc.tile_pool(name="msk", bufs=1) as mskp,
            tc.tile_pool(name="strp", bufs=1) as strp,
            tc.tile_pool(name="h1p", bufs=9) as h1p,
            tc.tile_pool(name="h2p", bufs=10) as h2p,
            tc.tile_pool(name="dram", bufs=1, space="DRAM") as dramp,
            tc.tile_pool(name="pz1", bufs=4, space="PSUM") as pz1,
            tc.tile_pool(name="pz2", bufs=2, space="PSUM") as pz2,
            tc.tile_pool(name="pz3", bufs=2, space="PSUM") as pz3,
        ):
            # ---- constants ----
            wstack = const.tile([128, 768], f16)
            nc.sync.dma_start(wstack[:], wstack_d.ap())
            w2t = const.tile([128, 128], f16)
            nc.sync.dma_start(w2t[:], w2t_d.ap())
            w3t = const.tile([128, 64], f16)
            nc.sync.dma_start(w3t[:], w3t_d.ap())
            b1c = const.tile([128, 1], f32)
            nc.sync.dma_start(b1c[:], b1_d.ap())
            b2c = const.tile([128, 1], f32)
            nc.sync.dma_start(b2c[:], b2_d.ap())
            b3c = const.tile([128, 1], f32)
            nc.sync.dma_start(b3c[:], b3_d.ap())

            # ---- load x bf16 (halo'd rows: buffer row r -> image row u*24 + r - 1) ----
            xf = xfp.tile([128, BL, RPU + 2, W], f16)
            nc.vector.memset(xf[0:32, :, 0:1, :], 0.0)
            nc.vector.memset(xf[96:128, :, 25:26, :], 0.0)
            for img in range(BL):
                for u in range(U):
                    lo = max(0, u * RPU - 1)
                    hi = min(H, u * RPU + RPU + 1)
                    rb0 = 1 - (u * RPU - lo)  # 0 normally; 1 for u==0
                    nc.sync.dma_start(
                        xf[u * 16:(u + 1) * 16, img, rb0:rb0 + (hi - lo), :],
                        x_d.ap()[img, :, lo:hi, :],
                    )

            # ---- update mask (host-computed {0,1}), broadcast over channels ----
            umasks = [mskp.tile([128, RPU, W], f16, tag=f"um{i}", name=f"um{i}") for i in range(BL)]
            for img in range(BL):
                for u in range(U):
                    src = fn_d.ap()[img, u * RPU:(u + 1) * RPU, :]
                    src = src.rearrange("a b -> (a b)").partition_broadcast(16)
                    nc.sync.dma_start(umasks[img][u * 16:(u + 1) * 16], src)

            # ---- PE warmup runway over the DMA/dw-build ramp ----
            zw = pz3.tile([128, TS], f32, tag="z3", name="zw")
            for _ in range(WARMN):
                nc.tensor.matmul(zw[:, :], w2t[:, :], wstack[:, 0:TS], start=True, stop=True)

            # ---- depthwise sobel builds (separable, pairwise sums) ----
            # img0 on DVE (fast ramp), img1 on Pool (overlaps img0 MLP)
            dwxs_t = [dwp.tile([128, RPU, W], f16, tag=f"dwx{i}", name=f"dwx{i}") for i in range(BL)]
            dwys_t = [dwp.tile([128, RPU, W], f16, tag=f"dwy{i}", name=f"dwy{i}") for i in range(BL)]
            RC = 6  # chunk rows

            def emit_build(img, eng):
                for r0 in range(0, RPU, RC):
                    # x buffer rows r0 .. r0+RC+2 cover image rows r0-1 .. r0+RC+1
                    ps = chk.tile([128, RC + 1, W], f16, tag="ps")
                    eng.tensor_add(
                        ps[:], xf[:, img, r0:r0 + RC + 1, :], xf[:, img, r0 + 1:r0 + RC + 2, :]
                    )
                    v1 = chk.tile([128, RC, W], f16, tag="v1")
                    eng.tensor_add(v1[:], ps[:, 0:RC, :], ps[:, 1:RC + 1, :])
                    v2 = chk.tile([128, RC, W], f16, tag="v2")
                    eng.tensor_sub(
                        v2[:], xf[:, img, r0 + 2:r0 + RC + 2, :], xf[:, img, r0:r0 + RC, :]
                    )
                    qs = chk.tile([128, RC, W], f16, tag="qs")
                    eng.tensor_add(qs[:, :, 0:191], v2[:, :, 0:191], v2[:, :, 1:192])
                    dxs = dwxs_t[img][:, r0:r0 + RC, :]
                    dys = dwys_t[img][:, r0:r0 + RC, :]
                    # dwx = v1[c+1] - v1[c-1]; borders zero-padded
                    eng.tensor_sub(dxs[:, :, 1:191], v1[:, :, 2:192], v1[:, :, 0:190])
                    eng.tensor_copy(dxs[:, :, 0:1], v1[:, :, 1:2])
                    eng.tensor_scalar_mul(dxs[:, :, 191:192], v1[:, :, 190:191], -1.0)
                    # dwy = qs[c-1] + qs[c]; borders: qs[0]+v2[0], qs[190]+v2[191]
                    eng.tensor_add(dys[:, :, 1:191], qs[:, :, 0:190], qs[:, :, 1:191])
                    eng.tensor_add(dys[:, :, 0:1], qs[:, :, 0:1], v2[:, :, 0:1])
                    eng.tensor_add(dys[:, :, 191:192], qs[:, :, 190:191], v2[:, :, 191:192])

            emit_build(0, nc.vector)
            emit_build(1, nc.gpsimd)

            # ---- pre-life maxpool (stripe layout: partition = img*64 + s, 3 rows each) ----
            xf_r = xf[:].rearrange("(u c) i r w -> u c i r w", c=16)

            al = strp.tile([128, 3, W], f16, tag="al_s")
            for img in range(BL):
                s = xf_r[:, 3, img, 1:25, :]
                s = s.rearrange("u (s r) w -> u s (r w)", r=3)
                nc.sync.dma_start(al[img * 64:(img + 1) * 64, :, :], s)
            pm = strp.tile([128, 3, 191], f16, tag="pm_s")
            nc.vector.tensor_max(pm[:], al[:, :, 0:191], al[:, :, 1:192])
            m1 = strp.tile([128, 3, W], f16, tag="m1_s")
            nc.vector.tensor_max(m1[:, :, 1:191], pm[:, :, 0:190], pm[:, :, 1:191])
            nc.vector.tensor_copy(m1[:, :, 0:1], pm[:, :, 0:1])
            nc.vector.tensor_copy(m1[:, :, 191:192], pm[:, :, 190:191])
            hh = strp.tile([128, 2, W], f16, tag="hh_s")
            nc.sync.dma_start(hh[1:128, 0, :], m1[0:127, 2, :])
            nc.sync.dma_start(hh[0:127, 1, :], m1[1:128, 0, :])
            m1_r = m1[:].rearrange("(i s) r w -> i s r w", s=64)
            hh_r = hh[:].rearrange("(i s) r w -> i s r w", s=64)
            nc.sync.dma_start(hh_r[:, 0, 0, :], m1_r[:, 0, 0, :])
            nc.sync.dma_start(hh_r[:, 63, 1, :], m1_r[:, 63, 2, :])
            pv = strp.tile([128, 2, W], f16, tag="pv_s")
            nc.vector.tensor_max(pv[:], m1[:, 0:2, :], m1[:, 1:3, :])
            m2pre = strp.tile([128, 3, W], f16, tag="m2_pre")
            nc.vector.tensor_max(m2pre[:, 0, :], pv[:, 0, :], hh[:, 0, :])
            nc.vector.tensor_max(m2pre[:, 1, :], pv[:, 0, :], pv[:, 1, :])
            nc.vector.tensor_max(m2pre[:, 2, :], pv[:, 1, :], hh[:, 1, :])

            # ---- MLP over tiles ----
            xf_flat = xf[:].rearrange("p i r w -> p (i r w)")
            dwx_flats = [t[:].rearrange("p r w -> p (r w)") for t in dwxs_t]
            dwy_flats = [t[:].rearrange("p r w -> p (r w)") for t in dwys_t]
            xnews = [mskp.tile([128, RPU, W], f16, tag=f"xn{i}", name=f"xn{i}") for i in range(BL)]
            xn_flats = [t[:].rearrange("p r w -> p (r w)") for t in xnews]
            um_flats = [t[:].rearrange("p r w -> p (r w)") for t in umasks]

            def mlp_tile(img, t):
                off = t * TS
                h1s = {}
                # L1: parity sets; k-rounds of 4 quadrant matmuls
                for par in range(2):
                    z1s = {u: pz1.tile([128, TS], f32, tag="z1", name=f"z1_{img}_{t}_{u}")
                           for u in (2 * q + par for q in range(4))}
                    for k in range(3):
                        for q in range(4):
                            u = 2 * q + par
                            base = 32 * q
                            blk = 2 * k + par
                            wv = wstack[base:base + 32, blk * 128:(blk + 1) * 128]
                            if k == 0:
                                flat = xf_flat
                                o = img * FHI + W + off
                            else:
                                flat = (dwx_flats if k == 1 else dwy_flats)[img]
                                o = off
                            nc.tensor.matmul(
                                z1s[u][:, :], wv, flat[base:base + 32, o:o + TS],
                                start=(k == 0), stop=(k == 2), tile_position=(base, 0),
                            )
                    for q in range(4):
                        u = 2 * q + par
                        h1g = h1p.tile([128, TS], f16, tag="h1", name=f"h1_{img}_{t}_{u}")
                        if EV1[u] == "a":
                            nc.scalar.activation(h1g[:], z1s[u][:], AF.Relu, bias=b1c[:])
                        else:
                            nc.vector.tensor_scalar(h1g[:], z1s[u][:], b1c[:], 0.0, OP.add, OP.max)
                        h1s[u] = h1g
                # L2: dense K=128 per unit
                h2s = {}
                for u in range(U):
                    z2g = pz2.tile([128, TS], f32, tag="z2", name=f"z2_{img}_{t}_{u}")
                    nc.tensor.matmul(z2g[:, :], w2t[:, :], h1s[u][:], start=True, stop=True)
                    h2g = h2p.tile([128, TS], f16, tag="h2", name=f"h2_{img}_{t}_{u}")
                    if EV2[u] == "a":
                        nc.scalar.activation(h2g[:], z2g[:], AF.Relu, bias=b2c[:])
                    else:
                        nc.vector.tensor_scalar(h2g[:], z2g[:], b2c[:], 0.0, OP.add, OP.max)
                    h2s[u] = h2g
                # L3: 2 rounds of 4 col-group matmuls
                z3 = pz3.tile([128, TS], f32, tag="z3", name=f"z3_{img}_{t}")
                for par in range(2):
                    for j in range(4):
                        nc.tensor.matmul(
                            z3[32 * j:32 * j + 32, :], w3t[:, 32 * par:32 * par + 32],
                            h2s[2 * j + par][:], start=(par == 0), stop=(par == 1),
                            tile_position=(0, 32 * j),
                        )
                # fused epilogue: x_new = (z3 + b3) * umask + x
                xn_sl = xn_flats[img][:, off:off + TS]
                nc.vector.scalar_tensor_tensor(
                    xn_sl, z3[:, :], b3c[:], um_flats[img][:, off:off + TS], OP.add, OP.mult
                )
                nc.gpsimd.tensor_add(xn_sl, xn_sl, xf_flat[:, img * FHI + W + off:img * FHI + W + off + TS])

            # ---- per-image post: post-life maxpool + life mask + store ----
            lifec_d = dramp.tile([128, 3 * W], f16)
            al_post = strp.tile([128, 3, W], f16, tag="al_p")
            pm_post = strp.tile([128, 3, 191], f16, tag="pm_p")
            m1_post = strp.tile([128, 3, W], f16, tag="m1_p")
            hh_post = strp.tile([128, 2, W], f16, tag="hh_p")
            pv_post = strp.tile([128, 2, W], f16, tag="pv_p")
            m2_post = strp.tile([128, 3, W], f16, tag="m2_p")
            lifec = strp.tile([128, 3 * W], f16, tag="lifec")

            def emit_post(img):
                xni = xnews[img]
                xn_r = xni[:].rearrange("(u c) r w -> u c r w", c=16)
                src = xn_r[:, 3, :, :].rearrange("u (s r) w -> u s (r w)", r=3)
                sl = slice(img * 64, (img + 1) * 64)
                nc.sync.dma_start(al_post[sl, :, :], src)
                nc.vector.tensor_max(pm_post[sl], al_post[sl, :, 0:191], al_post[sl, :, 1:192])
                nc.vector.tensor_max(m1_post[sl, :, 1:191], pm_post[sl, :, 0:190], pm_post[sl, :, 1:191])
                nc.vector.tensor_copy(m1_post[sl, :, 0:1], pm_post[sl, :, 0:1])
                nc.vector.tensor_copy(m1_post[sl, :, 191:192], pm_post[sl, :, 190:191])
                nc.sync.dma_start(hh_post[img * 64 + 1:(img + 1) * 64, 0, :], m1_post[img * 64:(img + 1) * 64 - 1, 2, :])
                nc.sync.dma_start(hh_post[img * 64:(img + 1) * 64 - 1, 1, :], m1_post[img * 64 + 1:(img + 1) * 64, 0, :])
                nc.sync.dma_start(hh_post[img * 64:img * 64 + 1, 0, :], m1_post[img * 64:img * 64 + 1, 0, :])
                nc.sync.dma_start(hh_post[(img + 1) * 64 - 1:(img + 1) * 64, 1, :], m1_post[(img + 1) * 64 - 1:(img + 1) * 64, 2, :])
                nc.vector.tensor_max(pv_post[sl], m1_post[sl, 0:2, :], m1_post[sl, 1:3, :])
                nc.vector.tensor_max(m2_post[sl, 0, :], pv_post[sl, 0, :], hh_post[sl, 0, :])
                nc.vector.tensor_max(m2_post[sl, 1, :], pv_post[sl, 0, :], pv_post[sl, 1, :])
                nc.vector.tensor_max(m2_post[sl, 2, :], pv_post[sl, 1, :], hh_post[sl, 1, :])

                nc.vector.tensor_tensor(
                    lifec[sl], m2pre[:].rearrange("p r w -> p (r w)")[sl],
                    m2_post[:].rearrange("p r w -> p (r w)")[sl], OP.min,
                )
                nc.vector.tensor_scalar(lifec[sl], lifec[sl], 0.1, None, OP.is_gt)

                # broadcast life over channels (bounce via DRAM)
                nc.sync.dma_start(lifec_d[sl], lifec[sl])
                life = mskp.tile([128, RPU, W], f16, tag=f"life{img}", name=f"life{img}")
                for u in range(U):
                    bsrc = lifec_d[img * 64 + 8 * u: img * 64 + 8 * u + 8, :]
                    bsrc = bsrc.rearrange("s w -> (s w)").partition_broadcast(16)
                    nc.sync.dma_start(life[u * 16:(u + 1) * 16], bsrc)

                # final mask multiply (bf16 out) + store
                out16 = mskp.tile([128, RPU, W], f16, tag=f"o16{img}", name=f"o16{img}")
                nc.vector.tensor_mul(out16[:], xni[:], life[:])
                for u in range(U):
                    nc.sync.dma_start(
                        out_d.ap()[img, :, u * RPU:(u + 1) * RPU, :],
                        out16[u * 16:(u + 1) * 16],
                    )

            # ---- driver ----
            for img in range(BL):
                for t in range(NT):
                    mlp_tile(img, t)
                emit_post(img)

    nc.compile()
    return nc


def host_prep(inputs):
    """Full inputs -> list of 8 per-core input dicts."""
    x = np.ascontiguousarray(inputs["x"], dtype=np.float32)
    fn = np.ascontiguousarray(inputs["fire_noise"], dtype=np.float32)
    w1 = np.asarray(inputs["w1"], np.float32)
    b1 = np.asarray(inputs["b1"], np.float32)
    w2 = np.asarray(inputs["w2"], np.float32)
    b2 = np.asarray(inputs["b2"], np.float32)
    w3 = np.asarray(inputs["w3"], np.float32)
    b3 = np.asarray(inputs["b3"], np.float32)

    w1a, w1b, w1c = w1[:, 0:16], w1[:, 16:32] / 8.0, w1[:, 32:48] / 8.0
    wstack = np.zeros((128, 768), ml_dtypes.bfloat16)
    for g in range(4):
        for k, comp in enumerate((w1a, w1b, w1c)):
            for par in range(2):
                blk = 2 * k + par
                r0 = 32 * g + 16 * par
                wstack[r0:r0 + 16, blk * 128:(blk + 1) * 128] = comp.T.astype(ml_dtypes.bfloat16)
    w2t = w2.T.astype(ml_dtypes.bfloat16)
    w3t = np.zeros((128, 64), ml_dtypes.bfloat16)
    w3t[:, 0:16] = w3.T.astype(ml_dtypes.bfloat16)
    w3t[:, 48:64] = w3.T.astype(ml_dtypes.bfloat16)
    b3col = np.tile(b3, U).reshape(128, 1).astype(np.float32)

    shared = {
        "wstack": wstack, "w2t": w2t, "w3t": w3t,
        "b1": b1.reshape(128, 1).astype(np.float32),
        "b2": b2.reshape(128, 1).astype(np.float32),
        "b3": b3col,
    }
    xh = x.astype(ml_dtypes.bfloat16)
    um = (fn[:, 0] <= 0.5).astype(ml_dtypes.bfloat16)
    in_maps = []
    for core in range(8):
        m = dict(shared)
        m["x"] = xh[2 * core:2 * core + 2]
        m["fn"] = um[2 * core:2 * core + 2]
        in_maps.append(m)
    return in_maps


_NC_CACHE = None


def kernel(**inputs):
    global _NC_CACHE
    from concourse.bass_utils import run_bass_kernel_spmd
    if _NC_CACHE is None:
        _NC_CACHE = build_nc()
    in_maps = host_prep(inputs)
    res = run_bass_kernel_spmd(_NC_CACHE, in_maps, core_ids=list(range(8)))
    return np.concatenate(
        [np.asarray(res.results[i]["out"]).astype(np.float32) for i in range(8)], axis=0
    )
